# revision 1
# baseline (speedup 1.0000x reference)
"""MinLSTM fused kernel for Trainium2 (8 NeuronCores, batch-parallel).

Contract: kernel(**inputs) takes the FULL inputs from setup_inputs()
  x    [8, 4096, 1024] f32
  w_gh [1024, 3072]    f32
and returns the FULL output next_cell [8, 4096, 1024] f32.

Strategy
--------
Data-parallel over batch: core b computes batch b. Per core:
  g = x[b] @ w_gh  (fp16 operands, fp32 PSUM accumulate; x centered at 0 and
  w scaled by 32 to stay in fp16 normal range — both undone exactly via the
  ScalarE activation's per-partition bias / scale arguments)
then the minLSTM recurrence in linear domain (no log/exp):
  f = sigmoid(g_f); i = sigmoid(g_i); th = g_h
  num = i+eps; s = (f+eps)+num
  a = 1 + (num+eps)/f          == exp(log_f_prime)
  b = s*th/num                 == exp(log_state)   (the a*eps term is < 4e-8
                                                    relative and is dropped)
  P = cumprod_t(a)             (VectorE tensor_tensor_scan along free dim)
  out = P*b
Layout: channels on partitions, T along the free dimension ([H, T] tiles), so
the T-scan maps onto the hardware scan. Device output is [H, T] per core; the
host transposes back when reassembling the [B, T, H] result.
"""

from contextlib import ExitStack

import numpy as np

import concourse.tile as tile
from concourse import bacc, mybir

F32 = mybir.dt.float32
F16 = mybir.dt.float16
AF = mybir.ActivationFunctionType
OP = mybir.AluOpType

B, T, H = 8, 4096, 1024
TC = 512
EPS = 1e-8
WSCALE = 32.0
N_CORES = 8

_prog_cache = {}


def _build():
    nc = bacc.Bacc("TRN2", target_bir_lowering=False, debug=False)
    KB = H // 128
    CB = H // 128
    NB = T // TC
    H3 = 3 * H
    MB = H3 // 128
    inv_ws = float(1.0 / WSCALE)

    xT = nc.dram_tensor("xT", [H, T], F16, kind="ExternalInput")
    w = nc.dram_tensor("w", [H, H3], F16, kind="ExternalInput")
    bias = nc.dram_tensor("bias", [H3], F32, kind="ExternalInput")
    out = nc.dram_tensor("out", [H, T], F32, kind="ExternalOutput")

    with ExitStack() as ctx:
        tc = ctx.enter_context(tile.TileContext(nc))
        singles = ctx.enter_context(tc.tile_pool(name="singles", bufs=1))
        xin = ctx.enter_context(tc.tile_pool(name="xin", bufs=3))
        ps = ctx.enter_context(tc.tile_pool(name="ps", bufs=2, space="PSUM"))
        ew = ctx.enter_context(tc.tile_pool(name="ew", bufs=3))
        pp = ctx.enter_context(tc.tile_pool(name="pp", bufs=2))
        outp = ctx.enter_context(tc.tile_pool(name="outp", bufs=3))

        w_sb = singles.tile([128, KB, H3], F16)
        nc.sync.dma_start(out=w_sb, in_=w.rearrange("(k p) m -> p k m", p=128))
        bias_sb = singles.tile([128, MB], F32)
        nc.sync.dma_start(out=bias_sb, in_=bias.rearrange("(j p) -> p j", p=128))

        xTr = xT.rearrange("(k p) t -> p k t", p=128)
        prevP = [None] * CB
        for n in range(NB):
            tsl = slice(n * TC, (n + 1) * TC)
            x_t = xin.tile([128, KB, TC], F16, tag="x")
            nc.sync.dma_start(out=x_t, in_=xTr[:, :, tsl])

            for c in range(CB):
                psf = ps.tile([128, TC], F32, tag="pf")
                psi = ps.tile([128, TC], F32, tag="pi")
                psh = ps.tile([128, TC], F32, tag="ph")
                for j, pt in ((c, psf), (CB + c, psi), (2 * CB + c, psh)):
                    for k in range(KB):
                        nc.tensor.matmul(pt,
                                         lhsT=w_sb[:, k, j * 128:(j + 1) * 128],
                                         rhs=x_t[:, k, :],
                                         start=(k == 0), stop=(k == KB - 1))

                f_t = ew.tile([128, TC], F32, tag="f")
                i_t = ew.tile([128, TC], F32, tag="i")
                th_t = ew.tile([128, TC], F32, tag="th")
                nc.scalar.activation(f_t, psf, AF.Sigmoid,
                                     bias=bias_sb[:, c:c + 1], scale=inv_ws)
                nc.scalar.activation(i_t, psi, AF.Sigmoid,
                                     bias=bias_sb[:, CB + c:CB + c + 1], scale=inv_ws)
                nc.scalar.activation(th_t, psh, AF.Identity,
                                     bias=bias_sb[:, 2 * CB + c:2 * CB + c + 1],
                                     scale=inv_ws)

                num_t = ew.tile([128, TC], F32, tag="num")
                nc.vector.tensor_scalar_add(num_t, i_t, EPS)
                s_t = ew.tile([128, TC], F32, tag="s")
                nc.vector.scalar_tensor_tensor(s_t, in0=f_t, scalar=EPS, in1=num_t,
                                               op0=OP.add, op1=OP.add)
                rden_t = ew.tile([128, TC], F32, tag="rden")
                nc.vector.reciprocal_approx_fast(rden_t, f_t)
                rnum_t = ew.tile([128, TC], F32, tag="rnum")
                nc.vector.reciprocal_approx_fast(rnum_t, num_t)
                u_t = ew.tile([128, TC], F32, tag="u")
                nc.vector.tensor_tensor(u_t, th_t, rnum_t, OP.mult)
                b_t = ew.tile([128, TC], F32, tag="b")
                nc.vector.tensor_tensor(b_t, s_t, u_t, OP.mult)

                # a = 1 + (num+eps)*rden rather than s*rden: the approx
                # reciprocal is ~1e-6 biased low and the scan integrates any
                # bias on a over all 4096 steps; keeping the reciprocal in the
                # small term makes that contribution negligible.
                t1_t = ew.tile([128, TC], F32, tag="t1")
                nc.vector.scalar_tensor_tensor(t1_t, in0=num_t, scalar=EPS,
                                               in1=rden_t, op0=OP.add, op1=OP.mult)
                a_t = ew.tile([128, TC], F32, tag="a")
                nc.vector.tensor_scalar_add(a_t, t1_t, 1.0)

                P_t = pp.tile([128, TC], F32, tag=f"P{c}")
                init = 1.0 if n == 0 else prevP[c][:, TC - 1:TC]
                nc.vector.tensor_tensor_scan(P_t, a_t, a_t, initial=init,
                                             op0=OP.mult, op1=OP.bypass)
                prevP[c] = P_t

                o_t = outp.tile([128, TC], F32, tag="o")
                nc.vector.tensor_tensor(o_t, P_t, b_t, OP.mult)
                nc.sync.dma_start(out=out[c * 128:(c + 1) * 128, tsl], in_=o_t)
    nc.finalize()
    return nc


def kernel(x, w_gh):
    assert x.shape == (B, T, H) and w_gh.shape == (H, 3 * H)
    if "nc" not in _prog_cache:
        _prog_cache["nc"] = _build()
    nc = _prog_cache["nc"]

    # host prep: center x at 0 (fold 0.5*colsum(w) into per-channel bias),
    # scale w by 32 to keep fp16 mantissas in the normal range
    bias = (0.5 * w_gh.astype(np.float64).sum(axis=0)).astype(np.float32)
    w16 = (w_gh.astype(np.float64) * WSCALE).astype(np.float16)
    xc = x.astype(np.float32) - np.float32(0.5)

    in_maps = []
    for b in range(B):
        xT16 = np.ascontiguousarray(xc[b].T).astype(np.float16)
        in_maps.append({"xT": xT16, "w": w16, "bias": bias})

    from concourse.bass_utils import run_bass_kernel_spmd
    res = run_bass_kernel_spmd(nc, in_maps, list(range(N_CORES)))

    out = np.empty((B, T, H), np.float32)
    for b in range(B):
        out[b] = res.results[b]["out"].T
    return out



# revision 2
# speedup vs baseline: 6934.1067x; 6934.1067x over previous
"""MinLSTM fused kernel for Trainium2 (8 NeuronCores, batch-parallel).

Contract: kernel(**inputs) takes the FULL inputs from setup_inputs()
  x    [8, 4096, 1024] f32
  w_gh [1024, 3072]    f32
and returns the FULL output next_cell [8, 4096, 1024] f32.

Strategy
--------
Data-parallel over batch: core b computes batch b. Per core the device gets
x[b] in its native [T, H] f16 layout (host does a single cast, no transpose);
the [H, T] operand needed by the matmul is produced on-device with xbar DMA
transposes. Compute:
  g = x[b] @ w_gh  (f16 operands scaled by 32, fp32 PSUM accumulate)
  f = sigmoid(g_f); i = sigmoid(g_i); th = g_h
  num = i+eps; s = (f+eps)+num
  a = 1 + (num+eps)/f          == exp(log_f_prime)
  bexp = s*th/num              == exp(log_state)   (the a*eps term is < 4e-8
                                                    relative and is dropped)
  P = cumprod_t(a)             (VectorE tensor_tensor_scan along free dim)
  out = P*bexp
Channels live on partitions, T on the free dim ([H, T] tiles) so the T-scan
maps onto the hardware scan. The result is PE-transposed back to [T, H] on
device and stored as f16 scaled by 1/64 (range fits f16); the host multiplies
by 64 while casting to f32.

The jitted executable, the device-resident weights and the (non-donated)
output staging buffers are built once per process and reused across calls,
so a warm kernel() call only ships x (f16) down and the f16 result back.
"""

from contextlib import ExitStack

import numpy as np

import concourse.tile as tile
from concourse import bacc, mybir
from concourse.masks import make_identity

F32 = mybir.dt.float32
F16 = mybir.dt.float16
AF = mybir.ActivationFunctionType
OP = mybir.AluOpType

B, T, H = 8, 4096, 1024
TC = 512
NB = T // TC          # 8 time blocks
KB = H // 128         # 8 contraction blocks
CB = H // 128         # 8 output-channel blocks
S = TC // 128         # 4 token sub-blocks per time block
H3 = 3 * H
EPS = 1e-8
WSCALE = 32.0
INV_WS = float(1.0 / WSCALE)
OSCALE = float(1.0 / 64.0)
INV_OS = 64.0
N_CORES = 8


def build(loop_n=None):
    nc = bacc.Bacc("TRN2", target_bir_lowering=False, debug=False)

    x = nc.dram_tensor("x", [T, H], F16, kind="ExternalInput")
    w = nc.dram_tensor("w", [H, H3], F16, kind="ExternalInput")
    out = nc.dram_tensor("out", [T, H], F16, kind="ExternalOutput")

    with ExitStack() as ctx:
        tc = ctx.enter_context(tile.TileContext(nc))
        singles = ctx.enter_context(tc.tile_pool(name="singles", bufs=1))
        xin = ctx.enter_context(tc.tile_pool(name="xin", bufs=3))
        ps = ctx.enter_context(tc.tile_pool(name="ps", bufs=2, space="PSUM"))
        psT = ctx.enter_context(tc.tile_pool(name="psT", bufs=2, space="PSUM"))
        ew = ctx.enter_context(tc.tile_pool(name="ew", bufs=2))
        pp = ctx.enter_context(tc.tile_pool(name="pp", bufs=2))
        op_ = ctx.enter_context(tc.tile_pool(name="op", bufs=2))
        outp = ctx.enter_context(tc.tile_pool(name="outp", bufs=2))

        w_sb = singles.tile([128, KB, H3], F16)
        nc.sync.dma_start(out=w_sb, in_=w.rearrange("(k p) m -> p k m", p=128))
        ident = singles.tile([128, 128], F32)
        make_identity(nc, ident)

        def body():
            prevP = [None] * CB
            o_hist = []  # [(n, [o_c tiles])] pending output transpose
            for n in range(NB):
                tsl = slice(n * TC, (n + 1) * TC)
                xT_t = xin.tile([128, KB, TC], F16, tag="xT")
                for k in range(KB):
                    nc.sync.dma_start_transpose(
                        xT_t[:, k, :], x[tsl, k * 128:(k + 1) * 128])

                o_tiles = []
                for c in range(CB):
                    psf = ps.tile([128, TC], F32, tag="pf")
                    psi = ps.tile([128, TC], F32, tag="pi")
                    psh = ps.tile([128, TC], F32, tag="ph")
                    for j, pt in ((c, psf), (CB + c, psi), (2 * CB + c, psh)):
                        for k in range(KB):
                            nc.tensor.matmul(pt,
                                             lhsT=w_sb[:, k, j * 128:(j + 1) * 128],
                                             rhs=xT_t[:, k, :],
                                             start=(k == 0), stop=(k == KB - 1))

                    f_t = ew.tile([128, TC], F32, tag="f")
                    i_t = ew.tile([128, TC], F32, tag="i")
                    th_t = ew.tile([128, TC], F32, tag="th")
                    nc.scalar.activation(f_t, psf, AF.Sigmoid, scale=INV_WS)
                    nc.scalar.activation(i_t, psi, AF.Sigmoid, scale=INV_WS)
                    nc.scalar.mul(th_t, psh, INV_WS)

                    num_t = ew.tile([128, TC], F32, tag="num")
                    nc.gpsimd.tensor_scalar_add(num_t, i_t, EPS)
                    rf_t = ew.tile([128, TC], F32, tag="rf")
                    nc.vector.reciprocal_approx_fast(rf_t, f_t)
                    rn_t = ew.tile([128, TC], F32, tag="rn")
                    nc.vector.reciprocal_approx_fast(rn_t, num_t)

                    # a = 1 + (num+eps)*rf; the approx reciprocal is ~1e-6
                    # biased and the scan integrates any bias on a over all
                    # 4096 steps, so keep the reciprocal in the small term.
                    t1_t = ew.tile([128, TC], F32, tag="t1")
                    nc.vector.scalar_tensor_tensor(t1_t, in0=num_t, scalar=EPS,
                                                   in1=rf_t, op0=OP.add,
                                                   op1=OP.mult)
                    a_t = ew.tile([128, TC], F32, tag="a")
                    nc.scalar.activation(a_t, t1_t, AF.Identity, bias=1.0)

                    s_t = ew.tile([128, TC], F32, tag="s")
                    nc.vector.scalar_tensor_tensor(s_t, in0=f_t, scalar=EPS,
                                                   in1=num_t, op0=OP.add,
                                                   op1=OP.add)
                    u_t = ew.tile([128, TC], F32, tag="u")
                    nc.vector.tensor_tensor(u_t, th_t, rn_t, OP.mult)
                    b_t = ew.tile([128, TC], F32, tag="b")
                    nc.vector.tensor_tensor(b_t, s_t, u_t, OP.mult)

                    P_t = pp.tile([128, TC], F32, tag=f"P{c}")
                    init = 1.0 if n == 0 else prevP[c][:, TC - 1:TC]
                    nc.vector.tensor_tensor_scan(P_t, a_t, a_t, initial=init,
                                                 op0=OP.mult, op1=OP.bypass)
                    prevP[c] = P_t

                    o_t = op_.tile([128, TC], F32, tag=f"o{c}")
                    nc.gpsimd.tensor_tensor(o_t, P_t, b_t, OP.mult)
                    o_tiles.append(o_t)

                o_hist.append((n, o_tiles))
                # transpose the previous block's outputs while this block's
                # matmuls still occupy the head of the PE queue
                if len(o_hist) > 1:
                    emit_out(o_hist.pop(0))
            emit_out(o_hist.pop(0))

        def emit_out(item):
            n, o_tiles = item
            for s in range(S):
                row = n * TC + s * 128
                ot = outp.tile([128, H], F16, tag="ot")
                for half in range(2):
                    pt = psT.tile([128, 512], F32, tag="pT")
                    for j in range(4):
                        c = half * 4 + j
                        nc.tensor.transpose(
                            pt[:, j * 128:(j + 1) * 128],
                            o_tiles[c][:, s * 128:(s + 1) * 128], ident)
                    nc.scalar.activation(ot[:, half * 512:(half + 1) * 512],
                                         pt, AF.Copy, scale=OSCALE)
                nc.sync.dma_start(out=out[row:row + 128, :], in_=ot)

        if loop_n is not None:
            with tc.For_i(0, loop_n, 1):
                body()
        else:
            body()
    nc.finalize()
    return nc


_ST: dict = {}


def _ensure_ready():
    if "f" in _ST:
        return _ST
    import jax
    from jax.sharding import Mesh, PartitionSpec, NamedSharding
    try:
        from jax.experimental.shard_map import shard_map
    except ImportError:
        from jax.shard_map import shard_map
    from concourse.bass2jax import (_bass_exec_p, install_neuronx_cc_hook,
                                    partition_id_tensor)

    nc = build()
    install_neuronx_cc_hook()
    devices = jax.devices()[:N_CORES]
    mesh = Mesh(np.asarray(devices), ("core",))
    sh = NamedSharding(mesh, PartitionSpec("core"))

    fn0 = nc.m.functions[0]
    in_names, out_names, out_avals = [], [], []
    for alloc in fn0.allocations:
        if not isinstance(alloc, mybir.MemoryLocationSet):
            continue
        name = alloc.memorylocations[0].name
        if alloc.kind == "ExternalInput":
            if nc.partition_id_tensor is None or name != nc.partition_id_tensor.name:
                in_names.append(name)
        elif alloc.kind == "ExternalOutput":
            out_names.append(name)
            out_avals.append(jax.core.ShapedArray(tuple(alloc.tensor_shape),
                                                  mybir.dt.np(alloc.dtype)))
    all_in = in_names + out_names
    if nc.partition_id_tensor is not None:
        all_in = all_in + [nc.partition_id_tensor.name]

    def _body(*args):
        operands = list(args)
        if nc.partition_id_tensor is not None:
            operands.append(partition_id_tensor())
        return tuple(_bass_exec_p.bind(
            *operands, out_avals=tuple(out_avals), in_names=tuple(all_in),
            out_names=tuple(out_names), lowering_input_output_aliases=(),
            sim_require_finite=True, sim_require_nnan=True, nc=nc))

    n_all = len(in_names) + len(out_names)
    f = jax.jit(shard_map(_body, mesh=mesh,
                          in_specs=(PartitionSpec("core"),) * n_all,
                          out_specs=(PartitionSpec("core"),) * len(out_names),
                          check_rep=False), keep_unused=True)

    _ST.update(dict(f=f, mesh=mesh, sh=sh, in_names=in_names,
                    out_names=out_names, jax=jax,
                    zeros_dev=jax.device_put(
                        np.zeros((N_CORES * T, H), np.float16), sh)))
    return _ST


def _cast_f16_threaded(x):
    out = np.empty(x.shape, np.float16)
    import concurrent.futures as cf
    with cf.ThreadPoolExecutor(max_workers=B) as ex:
        list(ex.map(lambda b: out[b].__setitem__(slice(None), x[b]), range(B)))
    return out


def kernel(x, w_gh):
    assert x.shape == (B, T, H) and w_gh.shape == (H, H3)
    st = _ensure_ready()
    jax, sh = st["jax"], st["sh"]

    w16 = (np.asarray(w_gh, np.float32) * np.float32(WSCALE)).astype(np.float16)
    if "w_host" not in st or not np.array_equal(st["w_host"], w16):
        st["w_host"] = w16
        st["w_dev"] = jax.device_put(np.broadcast_to(
            w16, (N_CORES, H, H3)).reshape(N_CORES * H, H3), sh)

    x16 = _cast_f16_threaded(np.asarray(x)).reshape(N_CORES * T, H)
    x_dev = jax.device_put(x16, sh)

    (out_dev,) = st["f"](x_dev, st["w_dev"], st["zeros_dev"])
    out16 = np.asarray(out_dev).reshape(B, T, H)

    res = np.empty((B, T, H), np.float32)
    import concurrent.futures as cf
    with cf.ThreadPoolExecutor(max_workers=B) as ex:
        list(ex.map(lambda b: np.multiply(out16[b], np.float32(INV_OS),
                                          out=res[b], dtype=np.float32),
                    range(B)))
    return res
